# revision 11
# baseline (speedup 1.0000x reference)
# Additive (Bahdanau) attention kernel for Trainium2, data-parallel over batch
# on 8 NeuronCores.
#
# Per core (2 batches):
#   qp = query @ Wq            [256q, 128h]   (kept transposed: qpT [h, q])
#   kp = key @ Wk              [256k, 128h]   (kept natural:    kp  [k, h])
#   score[k, q] = sum_h v[h] * tanh(qp[q, h] + kp[k, h])
#   attention = softmax_q(score);  context = attention @ value
#
# The [k, q] plane for each h is built on the TensorEngine as two K=128 fp16
# matmuls into PSUM: an identity pass that broadcasts kp[:, h] along q, plus a
# one-hot-row pass (T128) that broadcasts qpT[h, :] along k. ScalarE applies
# tanh (fp16 out). The v-weighted reduction over h is split: even-h planes
# fold into a score PSUM bank on the TensorEngine via scaled-identity
# accumulating matmuls (VI), odd-h planes fold into two fp16 accumulators on
# VectorE (two chains so pipe drains overlap). Softmax runs per k-tile with
# Exp+accum_out (scores are bounded by sum|v| <= 6.5, so no max subtraction).
# Both batches' transposes/projections are issued up front so the PE never
# stalls at the batch boundary.

import numpy as np

B_TOTAL = 16
N_CORES = 8
B_LOC = B_TOTAL // N_CORES
L = 256      # l_q == l_k
D = 256      # q_dim == k_dim
H = 128      # attention dim
VD = 128     # value dim
P = 128      # partitions

_cache = {}


def _build():
    from contextlib import ExitStack

    import concourse.bacc as bacc
    import concourse.mybir as mybir
    import concourse.tile as tile

    f32 = mybir.dt.float32
    f16 = mybir.dt.float16
    AF = mybir.ActivationFunctionType
    OP = mybir.AluOpType

    nc = bacc.Bacc("TRN2", target_bir_lowering=False, debug=False)

    q_d = nc.dram_tensor("query", [B_LOC, L, D], f32, kind="ExternalInput")
    k_d = nc.dram_tensor("key", [B_LOC, L, D], f32, kind="ExternalInput")
    val_d = nc.dram_tensor("value", [B_LOC, L, VD], f32, kind="ExternalInput")
    wq_d = nc.dram_tensor("Wq", [D, H], f32, kind="ExternalInput")
    wk_d = nc.dram_tensor("Wk", [D, H], f32, kind="ExternalInput")
    vv_d = nc.dram_tensor("v", [H], f32, kind="ExternalInput")
    ctx_d = nc.dram_tensor("context", [B_LOC, L, VD], f32, kind="ExternalOutput")
    attn_d = nc.dram_tensor("attention", [B_LOC, L, L], f32, kind="ExternalOutput")

    KT = L // P   # k tiles (2)
    QT = L // P   # q tiles (2)
    DC = D // P   # d chunks (2)
    HH = 2        # h planes per big PSUM tile (= HH banks)

    with tile.TileContext(nc) as tc, ExitStack() as ctx:
        singles = ctx.enter_context(tc.tile_pool(name="singles", bufs=1))
        io = ctx.enter_context(tc.tile_pool(name="io", bufs=2))
        tr = ctx.enter_context(tc.tile_pool(name="tr", bufs=2))
        proj = ctx.enter_context(tc.tile_pool(name="proj", bufs=2))
        tanhp = ctx.enter_context(tc.tile_pool(name="tanhp", bufs=3))
        accp = ctx.enter_context(tc.tile_pool(name="accp", bufs=2))
        soft = ctx.enter_context(tc.tile_pool(name="soft", bufs=2))
        outp = ctx.enter_context(tc.tile_pool(name="outp", bufs=2))
        # 6 banks of plane tiles (shared with prologue/epilogue via tag "ps")
        # + 2 banks of per-batch score accumulators = all 8 PSUM banks.
        psum_big = ctx.enter_context(tc.tile_pool(name="psum_big", bufs=3, space="PSUM"))
        psum_sc = ctx.enter_context(tc.tile_pool(name="psum_sc", bufs=2, space="PSUM"))

        # --- constants -----------------------------------------------------
        # delta[p, m] = p - m on GpSimd; identity via DVE is_equal (fast).
        delta = singles.tile([P, P], mybir.dt.int32, tag="delta")
        nc.gpsimd.iota(delta, pattern=[[-1, P]], base=0, channel_multiplier=1)
        ident16 = singles.tile([P, P], f16, tag="ident16")
        nc.vector.tensor_scalar(
            out=ident16, in0=delta, scalar1=0, scalar2=None, op0=OP.is_equal
        )

        # T128[p, h, k] = 1.0 iff p == h: lhsT slice T128[:, h, :] (contiguous,
        # fast weight load) is a K=128 stationary whose row h is all-ones; the
        # matmul broadcasts moving row h (a qpT row) to all 128 output
        # partitions while keeping the full PE array busy (K=32 variants
        # starve the HAM clock governor). Built on otherwise-idle GpSimd in
        # 32-h chunks so only the first chunk gates the main loop.
        t128 = singles.tile([P, P, P], f16, tag="t128")
        for c in range(4):
            chunk = t128[:, 32 * c:32 * (c + 1), :]
            nc.gpsimd.memset(chunk, 0.0)
            nc.gpsimd.affine_select(
                out=chunk, in_=chunk, compare_op=OP.not_equal, fill=1.0,
                base=-32 * c, pattern=[[-1, 32], [0, P]], channel_multiplier=1,
            )

        vb = singles.tile([P, H], f32, tag="vb")  # v broadcast across partitions
        nc.sync.dma_start(out=vb, in_=vv_d[:].unsqueeze(0).broadcast_to([P, H]))
        vb16 = singles.tile([P, H], f16, tag="vb16")
        nc.scalar.copy(out=vb16, in_=vb)

        # VI[p, h, k] = v[h] * (p == k): lhsT slice VI[:, h, :] is a scaled
        # identity; an accumulating matmul with it folds v_h * T_h into the
        # score PSUM bank (fp32) without touching VectorE. Chunked builds.
        vi = singles.tile([P, H, P], f16, tag="vi")
        for c in range(4):
            nc.vector.tensor_tensor(
                out=vi[:, 32 * c:32 * (c + 1), :],
                in0=ident16.unsqueeze(1).broadcast_to([P, 32, P]),
                in1=vb16[:, 32 * c:32 * (c + 1)].unsqueeze(2).broadcast_to([P, 32, P]),
                op=OP.mult,
            )

        wq_sb = singles.tile([P, DC, H], f32, tag="wq_sb")
        nc.sync.dma_start(out=wq_sb, in_=wq_d[:].rearrange("(c p) h -> p c h", p=P))
        wk_sb = singles.tile([P, DC, H], f32, tag="wk_sb")
        nc.sync.dma_start(out=wk_sb, in_=wk_d[:].rearrange("(c p) h -> p c h", p=P))
        wq16 = singles.tile([P, DC, H], f16, tag="wq")
        nc.scalar.copy(out=wq16, in_=wq_sb)
        wk16 = singles.tile([P, DC, H], f16, tag="wk")
        nc.scalar.copy(out=wk16, in_=wk_sb)

        def psum16(ap):
            # view the first bank of a big fp32 PSUM tile as a [P, 128] fp16
            # scratch for fp16 transposes
            return ap[:, 0, 0, :P].bitcast(f16)[:, :P]

        # --- prologue for all batches (software-pipelined) -----------------
        qpT16s, kp16s, val_sbs = [], [], []
        for b in range(B_LOC):
            q_sb = io.tile([P, QT, D], f32, tag="q_sb")
            nc.sync.dma_start(out=q_sb, in_=q_d[b].rearrange("(t p) d -> p t d", p=P))
            k_sb = io.tile([P, KT, D], f32, tag="k_sb")
            nc.sync.dma_start(out=k_sb, in_=k_d[b].rearrange("(t p) d -> p t d", p=P))
            val_sb = io.tile([P, QT, VD], f32, tag="val_sb")
            nc.sync.dma_start(out=val_sb, in_=val_d[b].rearrange("(t p) v -> p t v", p=P))
            val_sbs.append(val_sb)

            # fp16 copies of q/k (ScalarE), then fp16 PE transposes.
            q16 = io.tile([P, QT, D], f16, tag="q16")
            nc.scalar.copy(out=q16, in_=q_sb)
            k16 = io.tile([P, KT, D], f16, tag="k16")
            nc.scalar.copy(out=k16, in_=k_sb)

            qT16 = tr.tile([P, DC, L], f16, tag="qT")
            kT16 = tr.tile([P, DC, L], f16, tag="kT")
            for src, dst in ((q16, qT16), (k16, kT16)):
                for t in range(QT):
                    for dc in range(DC):
                        psT = psum_big.tile([P, HH, KT, L], f32, tag="ps")
                        nc.tensor.transpose(
                            psum16(psT),
                            src[:, t, dc * P:(dc + 1) * P],
                            ident16,
                        )
                        nc.vector.tensor_copy(
                            out=dst[:, dc, t * P:(t + 1) * P], in_=psum16(psT)
                        )

            # qpT[h, q] = sum_d Wq[d, h] * queryT[d, q]
            qpT_ps = psum_big.tile([P, HH, KT, L], f32, tag="ps")
            for dc in range(DC):
                nc.tensor.matmul(
                    qpT_ps[:, 0, 0, :],
                    lhsT=wq16[:, dc],
                    rhs=qT16[:, dc],
                    start=(dc == 0),
                    stop=(dc == DC - 1),
                )
            qpT16 = proj.tile([P, L], f16, tag="qpT16")
            nc.scalar.copy(out=qpT16, in_=qpT_ps[:, 0, 0, :])
            qpT16s.append(qpT16)

            # kp[k, h] = sum_d keyT[d, k] * Wk[d, h]
            kp16 = proj.tile([P, KT, H], f16, tag="kp16")
            for kt in range(KT):
                kp_ps = psum_big.tile([P, HH, KT, L], f32, tag="ps")
                for dc in range(DC):
                    nc.tensor.matmul(
                        kp_ps[:, 0, 0, :H],
                        lhsT=kT16[:, dc, kt * P:(kt + 1) * P],
                        rhs=wk16[:, dc],
                        start=(dc == 0),
                        stop=(dc == DC - 1),
                    )
                nc.scalar.copy(out=kp16[:, kt, :], in_=kp_ps[:, 0, 0, :H])
            kp16s.append(kp16)

        # --- main: score accumulation + softmax + context per batch --------
        for b in range(B_LOC):
            qpT16, kp16, val_sb = qpT16s[b], kp16s[b], val_sbs[b]

            acc_a = accp.tile([P, KT, L], f16, tag="acc_a")
            nc.vector.memset(acc_a, 0.0)
            acc_b = accp.tile([P, KT, L], f16, tag="acc_b")
            nc.vector.memset(acc_b, 0.0)
            score_ps = psum_sc.tile([P, KT, L], f32, tag="score")

            for hp in range(H // HH):
                ps = psum_big.tile([P, HH, KT, L], f32, tag="ps")
                T = tanhp.tile([P, HH, KT, L], f16, tag="T")
                for hh in range(HH):
                    h = HH * hp + hh
                    # plane[k, (kt, q)] = kp[kt*128+k, h]  (identity pass)
                    nc.tensor.matmul(
                        ps[:, hh],
                        lhsT=ident16,
                        rhs=kp16[:, :, h].unsqueeze(2).broadcast_to([P, KT, L]),
                        start=True,
                        stop=False,
                    )
                    # plane[k, (kt, q)] += qpT[h, q]  (one-hot row pass)
                    nc.tensor.matmul(
                        ps[:, hh],
                        lhsT=t128[:, h, :],
                        rhs=qpT16.unsqueeze(1).broadcast_to([P, KT, L]),
                        start=False,
                        stop=True,
                    )
                nc.scalar.activation(out=T, in_=ps, func=AF.Tanh)
                for hh in range(HH):
                    h = HH * hp + hh
                    if h % 2 == 0:
                        nc.tensor.matmul(
                            score_ps,
                            lhsT=vi[:, h, :],
                            rhs=T[:, hh],
                            start=(h == 0),
                            stop=(h == H - 2),
                        )
                    else:
                        acc = acc_a if (h % 4 == 1) else acc_b
                        nc.vector.scalar_tensor_tensor(
                            out=acc,
                            in0=T[:, hh],
                            scalar=vb[:, h:h + 1],
                            in1=acc,
                            op0=OP.mult,
                            op1=OP.add,
                        )

            scoref = soft.tile([P, KT, L], f32, tag="scoref")
            nc.vector.scalar_tensor_tensor(
                out=scoref, in0=acc_a, scalar=1.0, in1=acc_b,
                op0=OP.mult, op1=OP.add,
            )
            nc.vector.tensor_tensor(
                out=scoref, in0=scoref, in1=score_ps, op=OP.add
            )

            # --- softmax over q (free dim); scores bounded, skip max -------
            e = soft.tile([P, KT, L], f32, tag="e")
            sums = soft.tile([P, KT], f32, tag="sums")
            rec = soft.tile([P, KT], f32, tag="rec")
            for kt in range(KT):
                nc.scalar.activation(
                    out=e[:, kt],
                    in_=scoref[:, kt],
                    func=AF.Exp,
                    accum_out=sums[:, kt:kt + 1],
                )
            nc.vector.reciprocal(rec, sums)
            e16 = soft.tile([P, KT, L], f16, tag="e16")
            for kt in range(KT):
                nc.vector.tensor_scalar_mul(e[:, kt], e[:, kt], rec[:, kt:kt + 1])
                nc.sync.dma_start(
                    out=attn_d[b, kt * P:(kt + 1) * P, :], in_=e[:, kt]
                )
            nc.scalar.copy(out=e16, in_=e)

            # --- context = attention @ value (fp16 transposes/matmuls) -----
            attnT = outp.tile([P, QT, L], f16, tag="attnT")
            for kt in range(KT):
                for qc in range(QT):
                    psT = psum_big.tile([P, HH, KT, L], f32, tag="ps")
                    nc.tensor.transpose(
                        psum16(psT),
                        e16[:, kt, qc * P:(qc + 1) * P],
                        ident16,
                    )
                    nc.vector.tensor_copy(
                        out=attnT[:, qc, kt * P:(kt + 1) * P], in_=psum16(psT)
                    )
            val16 = outp.tile([P, QT, VD], f16, tag="val16")
            nc.scalar.copy(out=val16, in_=val_sb)
            for kt in range(KT):
                ctx_ps = psum_big.tile([P, HH, KT, L], f32, tag="ps")
                for qc in range(QT):
                    nc.tensor.matmul(
                        ctx_ps[:, 0, 0, :VD],
                        lhsT=attnT[:, qc, kt * P:(kt + 1) * P],
                        rhs=val16[:, qc],
                        start=(qc == 0),
                        stop=(qc == QT - 1),
                    )
                ctx_sb = outp.tile([P, VD], f32, tag="ctx_sb")
                nc.vector.tensor_copy(out=ctx_sb, in_=ctx_ps[:, 0, 0, :VD])
                nc.sync.dma_start(out=ctx_d[b, kt * P:(kt + 1) * P, :], in_=ctx_sb)

    nc.compile()
    return nc


def _get_nc():
    if "nc" not in _cache:
        _cache["nc"] = _build()
    return _cache["nc"]


def run_sharded(inputs, trace=False, trace_cores=None):
    from concourse.bass_utils import run_bass_kernel_spmd

    nc = _get_nc()
    q = np.ascontiguousarray(np.asarray(inputs["query"]), dtype=np.float32)
    k = np.ascontiguousarray(np.asarray(inputs["key"]), dtype=np.float32)
    val = np.ascontiguousarray(np.asarray(inputs["value"]), dtype=np.float32)
    wq = np.ascontiguousarray(np.asarray(inputs["Wq"]), dtype=np.float32)
    wk = np.ascontiguousarray(np.asarray(inputs["Wk"]), dtype=np.float32)
    vv = np.ascontiguousarray(np.asarray(inputs["v"]), dtype=np.float32)

    in_maps = []
    for c in range(N_CORES):
        sl = slice(c * B_LOC, (c + 1) * B_LOC)
        in_maps.append(
            {
                "query": np.ascontiguousarray(q[sl]),
                "key": np.ascontiguousarray(k[sl]),
                "value": np.ascontiguousarray(val[sl]),
                "Wq": wq,
                "Wk": wk,
                "v": vv,
            }
        )
    kwargs = {}
    if trace_cores is not None:
        kwargs["trace_cores"] = trace_cores
    res = run_bass_kernel_spmd(
        nc, in_maps, core_ids=list(range(N_CORES)), trace=trace, **kwargs
    )
    context = np.concatenate([r["context"] for r in res.results], axis=0)
    attention = np.concatenate([r["attention"] for r in res.results], axis=0)
    return (context, attention), res


def kernel(**inputs):
    (context, attention), _ = run_sharded(inputs, trace=False)
    return context, attention


if __name__ == "__main__":
    nc = _build()
    print("build + compile OK")


# revision 12
# speedup vs baseline: 1.0409x; 1.0409x over previous
# Additive (Bahdanau) attention kernel for Trainium2, data-parallel over batch
# on 8 NeuronCores.
#
# Per core (2 batches):
#   qp = query @ Wq            [256q, 128h]   (kept transposed: qpT [h, q])
#   kp = key @ Wk              [256k, 128h]   (kept natural:    kp  [k, h])
#   score[k, q] = sum_h v[h] * tanh(qp[q, h] + kp[k, h])
#   attention = softmax_q(score);  context = attention @ value
#
# The [k, q] plane for each h is built on the TensorEngine as two K=128 fp16
# matmuls into PSUM: an identity pass that broadcasts kp[:, h] along q, plus a
# one-hot-row pass (T128) that broadcasts qpT[h, :] along k. ScalarE applies
# tanh (fp16 out). The v-weighted reduction over h is split: even-h planes
# fold into a score PSUM bank on the TensorEngine via scaled-identity
# accumulating matmuls (VI), odd-h planes fold into two fp16 accumulators on
# VectorE (two chains so pipe drains overlap). Softmax runs per k-tile with
# Exp+accum_out (scores are bounded by sum|v| <= 6.5, so no max subtraction).
# Both batches' transposes/projections are issued up front so the PE never
# stalls at the batch boundary.

import numpy as np

B_TOTAL = 16
N_CORES = 8
B_LOC = B_TOTAL // N_CORES
L = 256      # l_q == l_k
D = 256      # q_dim == k_dim
H = 128      # attention dim
VD = 128     # value dim
P = 128      # partitions

_cache = {}


def _build():
    from contextlib import ExitStack

    import concourse.bacc as bacc
    import concourse.mybir as mybir
    import concourse.tile as tile

    f32 = mybir.dt.float32
    f16 = mybir.dt.float16
    AF = mybir.ActivationFunctionType
    OP = mybir.AluOpType

    nc = bacc.Bacc("TRN2", target_bir_lowering=False, debug=False)

    q_d = nc.dram_tensor("query", [B_LOC, L, D], f32, kind="ExternalInput")
    k_d = nc.dram_tensor("key", [B_LOC, L, D], f32, kind="ExternalInput")
    val_d = nc.dram_tensor("value", [B_LOC, L, VD], f32, kind="ExternalInput")
    wq_d = nc.dram_tensor("Wq", [D, H], f32, kind="ExternalInput")
    wk_d = nc.dram_tensor("Wk", [D, H], f32, kind="ExternalInput")
    vv_d = nc.dram_tensor("v", [H], f32, kind="ExternalInput")
    ctx_d = nc.dram_tensor("context", [B_LOC, L, VD], f32, kind="ExternalOutput")
    attn_d = nc.dram_tensor("attention", [B_LOC, L, L], f32, kind="ExternalOutput")

    KT = L // P   # k tiles (2)
    QT = L // P   # q tiles (2)
    DC = D // P   # d chunks (2)
    HH = 2        # h planes per big PSUM tile (= HH banks)

    with tile.TileContext(nc) as tc, ExitStack() as ctx:
        singles = ctx.enter_context(tc.tile_pool(name="singles", bufs=1))
        io = ctx.enter_context(tc.tile_pool(name="io", bufs=2))
        tr = ctx.enter_context(tc.tile_pool(name="tr", bufs=2))
        proj = ctx.enter_context(tc.tile_pool(name="proj", bufs=2))
        tanhp = ctx.enter_context(tc.tile_pool(name="tanhp", bufs=3))
        accp = ctx.enter_context(tc.tile_pool(name="accp", bufs=2))
        soft = ctx.enter_context(tc.tile_pool(name="soft", bufs=2))
        outp = ctx.enter_context(tc.tile_pool(name="outp", bufs=2))
        # 6 banks of plane tiles (shared with prologue/epilogue via tag "ps")
        # + 2 banks of per-batch score accumulators = all 8 PSUM banks.
        psum_big = ctx.enter_context(tc.tile_pool(name="psum_big", bufs=3, space="PSUM"))
        psum_sc = ctx.enter_context(tc.tile_pool(name="psum_sc", bufs=2, space="PSUM"))

        # --- constants -----------------------------------------------------
        # delta[p, m] = p - m on GpSimd; identity via DVE is_equal (fast).
        delta = singles.tile([P, P], mybir.dt.int32, tag="delta")
        nc.gpsimd.iota(delta, pattern=[[-1, P]], base=0, channel_multiplier=1)
        ident16 = singles.tile([P, P], f16, tag="ident16")
        nc.vector.tensor_scalar(
            out=ident16, in0=delta, scalar1=0, scalar2=None, op0=OP.is_equal
        )

        # T128[p, h, k] = 1.0 iff p == h: lhsT slice T128[:, h, :] (contiguous,
        # fast weight load) is a K=128 stationary whose row h is all-ones; the
        # matmul broadcasts moving row h (a qpT row) to all 128 output
        # partitions while keeping the full PE array busy (K=32 variants
        # starve the HAM clock governor). Built on otherwise-idle GpSimd in
        # 32-h chunks so only the first chunk gates the main loop.
        t128 = singles.tile([P, P, P], f16, tag="t128")
        for c in range(4):
            chunk = t128[:, 32 * c:32 * (c + 1), :]
            nc.gpsimd.memset(chunk, 0.0)
            nc.gpsimd.affine_select(
                out=chunk, in_=chunk, compare_op=OP.not_equal, fill=1.0,
                base=-32 * c, pattern=[[-1, 32], [0, P]], channel_multiplier=1,
            )

        vb = singles.tile([P, H], f32, tag="vb")  # v broadcast across partitions
        nc.sync.dma_start(out=vb, in_=vv_d[:].unsqueeze(0).broadcast_to([P, H]))
        vb16 = singles.tile([P, H], f16, tag="vb16")
        nc.scalar.copy(out=vb16, in_=vb)

        wq_sb = singles.tile([P, DC, H], f32, tag="wq_sb")
        nc.sync.dma_start(out=wq_sb, in_=wq_d[:].rearrange("(c p) h -> p c h", p=P))
        wk_sb = singles.tile([P, DC, H], f32, tag="wk_sb")
        nc.sync.dma_start(out=wk_sb, in_=wk_d[:].rearrange("(c p) h -> p c h", p=P))
        wq16 = singles.tile([P, DC, H], f16, tag="wq")
        nc.scalar.copy(out=wq16, in_=wq_sb)
        wk16 = singles.tile([P, DC, H], f16, tag="wk")
        nc.scalar.copy(out=wk16, in_=wk_sb)

        def psum16(ap):
            # view the first bank of a big fp32 PSUM tile as a [P, 128] fp16
            # scratch for fp16 transposes
            return ap[:, 0, 0, :P].bitcast(f16)[:, :P]

        # --- prologue for all batches (software-pipelined) -----------------
        qpT16s, kp16s, val_sbs = [], [], []
        for b in range(B_LOC):
            q_sb = io.tile([P, QT, D], f32, tag="q_sb")
            nc.sync.dma_start(out=q_sb, in_=q_d[b].rearrange("(t p) d -> p t d", p=P))
            k_sb = io.tile([P, KT, D], f32, tag="k_sb")
            nc.sync.dma_start(out=k_sb, in_=k_d[b].rearrange("(t p) d -> p t d", p=P))
            val_sb = io.tile([P, QT, VD], f32, tag="val_sb")
            nc.sync.dma_start(out=val_sb, in_=val_d[b].rearrange("(t p) v -> p t v", p=P))
            val_sbs.append(val_sb)

            # fp16 copies of q/k (ScalarE), then fp16 PE transposes.
            q16 = io.tile([P, QT, D], f16, tag="q16")
            nc.scalar.copy(out=q16, in_=q_sb)
            k16 = io.tile([P, KT, D], f16, tag="k16")
            nc.scalar.copy(out=k16, in_=k_sb)

            qT16 = tr.tile([P, DC, L], f16, tag="qT")
            kT16 = tr.tile([P, DC, L], f16, tag="kT")
            for src, dst in ((q16, qT16), (k16, kT16)):
                for t in range(QT):
                    for dc in range(DC):
                        psT = psum_big.tile([P, HH, KT, L], f32, tag="ps")
                        nc.tensor.transpose(
                            psum16(psT),
                            src[:, t, dc * P:(dc + 1) * P],
                            ident16,
                        )
                        nc.vector.tensor_copy(
                            out=dst[:, dc, t * P:(t + 1) * P], in_=psum16(psT)
                        )

            # qpT[h, q] = sum_d Wq[d, h] * queryT[d, q]
            qpT_ps = psum_big.tile([P, HH, KT, L], f32, tag="ps")
            for dc in range(DC):
                nc.tensor.matmul(
                    qpT_ps[:, 0, 0, :],
                    lhsT=wq16[:, dc],
                    rhs=qT16[:, dc],
                    start=(dc == 0),
                    stop=(dc == DC - 1),
                )
            qpT16 = proj.tile([P, L], f16, tag="qpT16")
            nc.scalar.copy(out=qpT16, in_=qpT_ps[:, 0, 0, :])
            qpT16s.append(qpT16)

            # kp[k, h] = sum_d keyT[d, k] * Wk[d, h]
            kp16 = proj.tile([P, KT, H], f16, tag="kp16")
            for kt in range(KT):
                kp_ps = psum_big.tile([P, HH, KT, L], f32, tag="ps")
                for dc in range(DC):
                    nc.tensor.matmul(
                        kp_ps[:, 0, 0, :H],
                        lhsT=kT16[:, dc, kt * P:(kt + 1) * P],
                        rhs=wk16[:, dc],
                        start=(dc == 0),
                        stop=(dc == DC - 1),
                    )
                nc.scalar.copy(out=kp16[:, kt, :], in_=kp_ps[:, 0, 0, :H])
            kp16s.append(kp16)

        # VI[p, h, k] = v[h] * (p == k): lhsT slice VI[:, h, :] is a scaled
        # identity; an accumulating matmul with it folds v_h * T_h into the
        # score PSUM bank (fp32) without touching VectorE. Emitted after the
        # prologue so its VectorE time doesn't starve the transpose copies.
        vi = singles.tile([P, H, P], f16, tag="vi")
        for c in range(4):
            nc.vector.tensor_tensor(
                out=vi[:, 32 * c:32 * (c + 1), :],
                in0=ident16.unsqueeze(1).broadcast_to([P, 32, P]),
                in1=vb16[:, 32 * c:32 * (c + 1)].unsqueeze(2).broadcast_to([P, 32, P]),
                op=OP.mult,
            )

        # --- main: score accumulation + softmax + context per batch --------
        state = {}

        def emit_planes(b):
            qpT16, kp16 = qpT16s[b], kp16s[b]
            acc_a = accp.tile([P, KT, L], f16, tag="acc_a")
            nc.vector.memset(acc_a, 0.0)
            acc_b = accp.tile([P, KT, L], f16, tag="acc_b")
            nc.vector.memset(acc_b, 0.0)
            score_ps = psum_sc.tile([P, KT, L], f32, tag="score")
            state[b] = (acc_a, acc_b, score_ps)

            for hp in range(H // HH):
                if b == 1 and hp == 20:
                    emit_softmax_ctx(0)
                ps = psum_big.tile([P, HH, KT, L], f32, tag="ps")
                T = tanhp.tile([P, HH, KT, L], f16, tag="T")
                for hh in range(HH):
                    h = HH * hp + hh
                    # plane[k, (kt, q)] = kp[kt*128+k, h]  (identity pass)
                    nc.tensor.matmul(
                        ps[:, hh],
                        lhsT=ident16,
                        rhs=kp16[:, :, h].unsqueeze(2).broadcast_to([P, KT, L]),
                        start=True,
                        stop=False,
                    )
                    # plane[k, (kt, q)] += qpT[h, q]  (one-hot row pass)
                    nc.tensor.matmul(
                        ps[:, hh],
                        lhsT=t128[:, h, :],
                        rhs=qpT16.unsqueeze(1).broadcast_to([P, KT, L]),
                        start=False,
                        stop=True,
                    )
                nc.scalar.activation(out=T, in_=ps, func=AF.Tanh)
                for hh in range(HH):
                    h = HH * hp + hh
                    if h % 2 == 0:
                        nc.tensor.matmul(
                            score_ps,
                            lhsT=vi[:, h, :],
                            rhs=T[:, hh],
                            start=(h == 0),
                            stop=(h == H - 2),
                        )
                    else:
                        acc = acc_a if (h % 4 == 1) else acc_b
                        nc.vector.scalar_tensor_tensor(
                            out=acc,
                            in0=T[:, hh],
                            scalar=vb[:, h:h + 1],
                            in1=acc,
                            op0=OP.mult,
                            op1=OP.add,
                        )

        def emit_softmax_ctx(b):
            acc_a, acc_b, score_ps = state[b]
            val_sb = val_sbs[b]
            scoref = soft.tile([P, KT, L], f32, tag="scoref")
            nc.vector.scalar_tensor_tensor(
                out=scoref, in0=acc_a, scalar=1.0, in1=acc_b,
                op0=OP.mult, op1=OP.add,
            )
            nc.vector.tensor_tensor(
                out=scoref, in0=scoref, in1=score_ps, op=OP.add
            )

            # softmax over q (free dim); scores bounded, skip max. The exp is
            # left unnormalized for the context path (folded via rec at the
            # end); only the attention output gets the explicit 1/sum scale.
            e = soft.tile([P, KT, L], f32, tag="e")
            sums = soft.tile([P, KT], f32, tag="sums")
            rec = soft.tile([P, KT], f32, tag="rec")
            for kt in range(KT):
                nc.scalar.activation(
                    out=e[:, kt],
                    in_=scoref[:, kt],
                    func=AF.Exp,
                    accum_out=sums[:, kt:kt + 1],
                )
            nc.vector.reciprocal(rec, sums)
            e16 = soft.tile([P, KT, L], f16, tag="e16")
            nc.scalar.copy(out=e16, in_=e)  # unnormalized, for context
            attn = soft.tile([P, KT, L], f32, tag="attn")
            for kt in range(KT):
                nc.vector.tensor_scalar_mul(attn[:, kt], e[:, kt], rec[:, kt:kt + 1])
                nc.sync.dma_start(
                    out=attn_d[b, kt * P:(kt + 1) * P, :], in_=attn[:, kt]
                )

            # context = softmax(score) @ value, via fp16 transposes/matmuls on
            # the unnormalized exp; 1/sum applied on the PSUM evacuation.
            attnT = outp.tile([P, QT, L], f16, tag="attnT")
            for kt in range(KT):
                for qc in range(QT):
                    psT = psum_big.tile([P, HH, KT, L], f32, tag="ps")
                    nc.tensor.transpose(
                        psum16(psT),
                        e16[:, kt, qc * P:(qc + 1) * P],
                        ident16,
                    )
                    nc.vector.tensor_copy(
                        out=attnT[:, qc, kt * P:(kt + 1) * P], in_=psum16(psT)
                    )
            val16 = outp.tile([P, QT, VD], f16, tag="val16")
            nc.scalar.copy(out=val16, in_=val_sbs[b])
            for kt in range(KT):
                ctx_ps = psum_big.tile([P, HH, KT, L], f32, tag="ps")
                for qc in range(QT):
                    nc.tensor.matmul(
                        ctx_ps[:, 0, 0, :VD],
                        lhsT=attnT[:, qc, kt * P:(kt + 1) * P],
                        rhs=val16[:, qc],
                        start=(qc == 0),
                        stop=(qc == QT - 1),
                    )
                ctx_sb = outp.tile([P, VD], f32, tag="ctx_sb")
                nc.vector.tensor_scalar_mul(ctx_sb, ctx_ps[:, 0, 0, :VD], rec[:, kt:kt + 1])
                nc.sync.dma_start(out=ctx_d[b, kt * P:(kt + 1) * P, :], in_=ctx_sb)

        emit_planes(0)
        emit_planes(1)
        emit_softmax_ctx(1)

    nc.compile()
    return nc


def _get_nc():
    if "nc" not in _cache:
        _cache["nc"] = _build()
    return _cache["nc"]


def run_sharded(inputs, trace=False, trace_cores=None):
    from concourse.bass_utils import run_bass_kernel_spmd

    nc = _get_nc()
    q = np.ascontiguousarray(np.asarray(inputs["query"]), dtype=np.float32)
    k = np.ascontiguousarray(np.asarray(inputs["key"]), dtype=np.float32)
    val = np.ascontiguousarray(np.asarray(inputs["value"]), dtype=np.float32)
    wq = np.ascontiguousarray(np.asarray(inputs["Wq"]), dtype=np.float32)
    wk = np.ascontiguousarray(np.asarray(inputs["Wk"]), dtype=np.float32)
    vv = np.ascontiguousarray(np.asarray(inputs["v"]), dtype=np.float32)

    in_maps = []
    for c in range(N_CORES):
        sl = slice(c * B_LOC, (c + 1) * B_LOC)
        in_maps.append(
            {
                "query": np.ascontiguousarray(q[sl]),
                "key": np.ascontiguousarray(k[sl]),
                "value": np.ascontiguousarray(val[sl]),
                "Wq": wq,
                "Wk": wk,
                "v": vv,
            }
        )
    kwargs = {}
    if trace_cores is not None:
        kwargs["trace_cores"] = trace_cores
    res = run_bass_kernel_spmd(
        nc, in_maps, core_ids=list(range(N_CORES)), trace=trace, **kwargs
    )
    context = np.concatenate([r["context"] for r in res.results], axis=0)
    attention = np.concatenate([r["attention"] for r in res.results], axis=0)
    return (context, attention), res


def kernel(**inputs):
    (context, attention), _ = run_sharded(inputs, trace=False)
    return context, attention


if __name__ == "__main__":
    nc = _build()
    print("build + compile OK")


# revision 13
# speedup vs baseline: 1.0553x; 1.0138x over previous
# Additive (Bahdanau) attention kernel for Trainium2, data-parallel over batch
# on 8 NeuronCores.
#
# Per core (2 batches):
#   qp = query @ Wq            [256q, 128h]   (kept transposed: qpT [h, q])
#   kp = key @ Wk              [256k, 128h]   (kept natural:    kp  [k, h])
#   score[k, q] = sum_h v[h] * tanh(qp[q, h] + kp[k, h])
#   attention = softmax_q(score);  context = attention @ value
#
# The [k, q] plane for each h is built on the TensorEngine as two K=128 fp16
# matmuls into PSUM: an identity pass that broadcasts kp[:, h] along q, plus a
# one-hot-row pass (T128) that broadcasts qpT[h, :] along k. ScalarE applies
# tanh (fp16 out). The v-weighted reduction over h is split: even-h planes
# fold into a score PSUM bank on the TensorEngine via scaled-identity
# accumulating matmuls (VI), odd-h planes fold into two fp16 accumulators on
# VectorE (two chains so pipe drains overlap). Softmax runs per k-tile with
# Exp+accum_out (scores are bounded by sum|v| <= 6.5, so no max subtraction).
# Both batches' transposes/projections are issued up front so the PE never
# stalls at the batch boundary.

import numpy as np

B_TOTAL = 16
N_CORES = 8
B_LOC = B_TOTAL // N_CORES
L = 256      # l_q == l_k
D = 256      # q_dim == k_dim
H = 128      # attention dim
VD = 128     # value dim
P = 128      # partitions

_cache = {}


def _build():
    from contextlib import ExitStack

    import concourse.bacc as bacc
    import concourse.mybir as mybir
    import concourse.tile as tile

    f32 = mybir.dt.float32
    f16 = mybir.dt.float16
    AF = mybir.ActivationFunctionType
    OP = mybir.AluOpType

    nc = bacc.Bacc("TRN2", target_bir_lowering=False, debug=False)

    q_d = nc.dram_tensor("query", [B_LOC, L, D], f32, kind="ExternalInput")
    k_d = nc.dram_tensor("key", [B_LOC, L, D], f32, kind="ExternalInput")
    val_d = nc.dram_tensor("value", [B_LOC, L, VD], f32, kind="ExternalInput")
    wq_d = nc.dram_tensor("Wq", [D, H], f32, kind="ExternalInput")
    wk_d = nc.dram_tensor("Wk", [D, H], f32, kind="ExternalInput")
    vv_d = nc.dram_tensor("v", [H], f32, kind="ExternalInput")
    ctx_d = nc.dram_tensor("context", [B_LOC, L, VD], f32, kind="ExternalOutput")
    attn_d = nc.dram_tensor("attention", [B_LOC, L, L], f32, kind="ExternalOutput")

    KT = L // P   # k tiles (2)
    QT = L // P   # q tiles (2)
    DC = D // P   # d chunks (2)
    HH = 2        # h planes per big PSUM tile (= HH banks)

    with tile.TileContext(nc) as tc, ExitStack() as ctx:
        singles = ctx.enter_context(tc.tile_pool(name="singles", bufs=1))
        io = ctx.enter_context(tc.tile_pool(name="io", bufs=2))
        tr = ctx.enter_context(tc.tile_pool(name="tr", bufs=2))
        proj = ctx.enter_context(tc.tile_pool(name="proj", bufs=2))
        tanhp = ctx.enter_context(tc.tile_pool(name="tanhp", bufs=3))
        accp = ctx.enter_context(tc.tile_pool(name="accp", bufs=2))
        soft = ctx.enter_context(tc.tile_pool(name="soft", bufs=2))
        outp = ctx.enter_context(tc.tile_pool(name="outp", bufs=2))
        # 6 banks of plane tiles (shared with prologue/epilogue via tag "ps")
        # + 2 banks of per-batch score accumulators = all 8 PSUM banks.
        psum_big = ctx.enter_context(tc.tile_pool(name="psum_big", bufs=3, space="PSUM"))
        psum_sc = ctx.enter_context(tc.tile_pool(name="psum_sc", bufs=2, space="PSUM"))

        # --- constants -----------------------------------------------------
        # delta[p, m] = p - m on GpSimd; identity via DVE is_equal (fast).
        delta = singles.tile([P, P], mybir.dt.int32, tag="delta")
        nc.gpsimd.iota(delta, pattern=[[-1, P]], base=0, channel_multiplier=1)
        ident16 = singles.tile([P, P], f16, tag="ident16")
        nc.vector.tensor_scalar(
            out=ident16, in0=delta, scalar1=0, scalar2=None, op0=OP.is_equal
        )
        ident32 = singles.tile([P, P], f32, tag="ident32")
        nc.vector.tensor_scalar(
            out=ident32, in0=delta, scalar1=0, scalar2=None, op0=OP.is_equal
        )

        # T128[p, h, k] = 1.0 iff p == h: lhsT slice T128[:, h, :] (contiguous,
        # fast weight load) is a K=128 stationary whose row h is all-ones; the
        # matmul broadcasts moving row h (a qpT row) to all 128 output
        # partitions while keeping the full PE array busy (K=32 variants
        # starve the HAM clock governor). Built on otherwise-idle GpSimd in
        # 32-h chunks so only the first chunk gates the main loop.
        t128 = singles.tile([P, P, P], f16, tag="t128")
        for c in range(4):
            chunk = t128[:, 32 * c:32 * (c + 1), :]
            nc.gpsimd.memset(chunk, 0.0)
            nc.gpsimd.affine_select(
                out=chunk, in_=chunk, compare_op=OP.not_equal, fill=1.0,
                base=-32 * c, pattern=[[-1, 32], [0, P]], channel_multiplier=1,
            )

        vb = singles.tile([P, H], f32, tag="vb")  # v broadcast across partitions
        nc.sync.dma_start(out=vb, in_=vv_d[:].unsqueeze(0).broadcast_to([P, H]))
        vb16 = singles.tile([P, H], f16, tag="vb16")
        nc.scalar.copy(out=vb16, in_=vb)

        wq_sb = singles.tile([P, DC, H], f32, tag="wq_sb")
        nc.sync.dma_start(out=wq_sb, in_=wq_d[:].rearrange("(c p) h -> p c h", p=P))
        wk_sb = singles.tile([P, DC, H], f32, tag="wk_sb")
        nc.sync.dma_start(out=wk_sb, in_=wk_d[:].rearrange("(c p) h -> p c h", p=P))
        wq16 = singles.tile([P, DC, H], f16, tag="wq")
        nc.scalar.copy(out=wq16, in_=wq_sb)
        wk16 = singles.tile([P, DC, H], f16, tag="wk")
        nc.scalar.copy(out=wk16, in_=wk_sb)

        # --- prologue for all batches (software-pipelined) -----------------
        qpT16s, kp16s, val_sbs = [], [], []
        for b in range(B_LOC):
            q_sb = io.tile([P, QT, D], f32, tag="q_sb")
            nc.sync.dma_start(out=q_sb, in_=q_d[b].rearrange("(t p) d -> p t d", p=P))
            k_sb = io.tile([P, KT, D], f32, tag="k_sb")
            nc.sync.dma_start(out=k_sb, in_=k_d[b].rearrange("(t p) d -> p t d", p=P))
            val_sb = io.tile([P, QT, VD], f32, tag="val_sb")
            nc.sync.dma_start(out=val_sb, in_=val_d[b].rearrange("(t p) v -> p t v", p=P))
            val_sbs.append(val_sb)

            # fp32 PE transposes; fp32->fp16 conversion happens in the
            # ScalarE PSUM-evacuation copy (shortens the startup chain).
            qT16 = tr.tile([P, DC, L], f16, tag="qT")
            kT16 = tr.tile([P, DC, L], f16, tag="kT")
            for src, dst in ((q_sb, qT16), (k_sb, kT16)):
                for t in range(QT):
                    for dc in range(DC):
                        psT = psum_big.tile([P, HH, KT, L], f32, tag="ps")
                        nc.tensor.transpose(
                            psT[:, 0, 0, :P],
                            src[:, t, dc * P:(dc + 1) * P],
                            ident32,
                        )
                        nc.scalar.copy(
                            out=dst[:, dc, t * P:(t + 1) * P], in_=psT[:, 0, 0, :P]
                        )

            # qpT[h, q] = sum_d Wq[d, h] * queryT[d, q]
            qpT_ps = psum_big.tile([P, HH, KT, L], f32, tag="ps")
            for dc in range(DC):
                nc.tensor.matmul(
                    qpT_ps[:, 0, 0, :],
                    lhsT=wq16[:, dc],
                    rhs=qT16[:, dc],
                    start=(dc == 0),
                    stop=(dc == DC - 1),
                )
            qpT16 = proj.tile([P, L], f16, tag="qpT16")
            nc.scalar.copy(out=qpT16, in_=qpT_ps[:, 0, 0, :])
            qpT16s.append(qpT16)

            # kp[k, h] = sum_d keyT[d, k] * Wk[d, h]
            kp16 = proj.tile([P, KT, H], f16, tag="kp16")
            for kt in range(KT):
                kp_ps = psum_big.tile([P, HH, KT, L], f32, tag="ps")
                for dc in range(DC):
                    nc.tensor.matmul(
                        kp_ps[:, 0, 0, :H],
                        lhsT=kT16[:, dc, kt * P:(kt + 1) * P],
                        rhs=wk16[:, dc],
                        start=(dc == 0),
                        stop=(dc == DC - 1),
                    )
                nc.scalar.copy(out=kp16[:, kt, :], in_=kp_ps[:, 0, 0, :H])
            kp16s.append(kp16)

        # VI[p, h, k] = v[h] * (p == k): lhsT slice VI[:, h, :] is a scaled
        # identity; an accumulating matmul with it folds v_h * T_h into the
        # score PSUM bank (fp32) without touching VectorE. Emitted after the
        # prologue so its VectorE time doesn't starve the transpose copies.
        vi = singles.tile([P, H, P], f16, tag="vi")
        for c in range(4):
            nc.vector.tensor_tensor(
                out=vi[:, 32 * c:32 * (c + 1), :],
                in0=ident16.unsqueeze(1).broadcast_to([P, 32, P]),
                in1=vb16[:, 32 * c:32 * (c + 1)].unsqueeze(2).broadcast_to([P, 32, P]),
                op=OP.mult,
            )

        # --- main: score accumulation + softmax + context per batch --------
        state = {}

        def emit_planes(b):
            qpT16, kp16 = qpT16s[b], kp16s[b]
            acc_a = accp.tile([P, KT, L], f16, tag="acc_a")
            nc.vector.memset(acc_a, 0.0)
            acc_b = accp.tile([P, KT, L], f16, tag="acc_b")
            nc.vector.memset(acc_b, 0.0)
            score_ps = psum_sc.tile([P, KT, L], f32, tag="score")
            state[b] = (acc_a, acc_b, score_ps)

            for hp in range(H // HH):
                if b == 1 and hp == 20:
                    emit_softmax_ctx(0)
                ps = psum_big.tile([P, HH, KT, L], f32, tag="ps")
                T = tanhp.tile([P, HH, KT, L], f16, tag="T")
                for hh in range(HH):
                    h = HH * hp + hh
                    # plane[k, (kt, q)] = kp[kt*128+k, h]  (identity pass)
                    nc.tensor.matmul(
                        ps[:, hh],
                        lhsT=ident16,
                        rhs=kp16[:, :, h].unsqueeze(2).broadcast_to([P, KT, L]),
                        start=True,
                        stop=False,
                    )
                    # plane[k, (kt, q)] += qpT[h, q]  (one-hot row pass)
                    nc.tensor.matmul(
                        ps[:, hh],
                        lhsT=t128[:, h, :],
                        rhs=qpT16.unsqueeze(1).broadcast_to([P, KT, L]),
                        start=False,
                        stop=True,
                    )
                nc.scalar.activation(out=T, in_=ps, func=AF.Tanh)
                for hh in range(HH):
                    h = HH * hp + hh
                    if h % 2 == 0:
                        nc.tensor.matmul(
                            score_ps,
                            lhsT=vi[:, h, :],
                            rhs=T[:, hh],
                            start=(h == 0),
                            stop=(h == H - 2),
                        )
                    else:
                        acc = acc_a if (h % 4 == 1) else acc_b
                        nc.vector.scalar_tensor_tensor(
                            out=acc,
                            in0=T[:, hh],
                            scalar=vb[:, h:h + 1],
                            in1=acc,
                            op0=OP.mult,
                            op1=OP.add,
                        )

        def emit_softmax_ctx(b):
            acc_a, acc_b, score_ps = state[b]
            val_sb = val_sbs[b]
            scoref = soft.tile([P, KT, L], f32, tag="scoref")
            nc.vector.scalar_tensor_tensor(
                out=scoref, in0=acc_a, scalar=1.0, in1=acc_b,
                op0=OP.mult, op1=OP.add,
            )
            nc.vector.tensor_tensor(
                out=scoref, in0=scoref, in1=score_ps, op=OP.add
            )

            # softmax over q (free dim); scores bounded, skip max. The exp is
            # left unnormalized for the context path (folded via rec at the
            # end); only the attention output gets the explicit 1/sum scale.
            e = soft.tile([P, KT, L], f32, tag="e")
            sums = soft.tile([P, KT], f32, tag="sums")
            rec = soft.tile([P, KT], f32, tag="rec")
            for kt in range(KT):
                nc.scalar.activation(
                    out=e[:, kt],
                    in_=scoref[:, kt],
                    func=AF.Exp,
                    accum_out=sums[:, kt:kt + 1],
                )
            nc.vector.reciprocal(rec, sums)
            attn = soft.tile([P, KT, L], f32, tag="attn")
            for kt in range(KT):
                nc.vector.tensor_scalar_mul(attn[:, kt], e[:, kt], rec[:, kt:kt + 1])
                nc.sync.dma_start(
                    out=attn_d[b, kt * P:(kt + 1) * P, :], in_=attn[:, kt]
                )

            # context = softmax(score) @ value, via fp16 transposes/matmuls on
            # the unnormalized exp; 1/sum applied on the PSUM evacuation.
            attnT = outp.tile([P, QT, L], f16, tag="attnT")
            for kt in range(KT):
                for qc in range(QT):
                    psT = psum_big.tile([P, HH, KT, L], f32, tag="ps")
                    nc.tensor.transpose(
                        psT[:, 0, 0, :P],
                        e[:, kt, qc * P:(qc + 1) * P],
                        ident32,
                    )
                    nc.scalar.copy(
                        out=attnT[:, qc, kt * P:(kt + 1) * P], in_=psT[:, 0, 0, :P]
                    )
            val16 = outp.tile([P, QT, VD], f16, tag="val16")
            nc.scalar.copy(out=val16, in_=val_sbs[b])
            for kt in range(KT):
                ctx_ps = psum_big.tile([P, HH, KT, L], f32, tag="ps")
                for qc in range(QT):
                    nc.tensor.matmul(
                        ctx_ps[:, 0, 0, :VD],
                        lhsT=attnT[:, qc, kt * P:(kt + 1) * P],
                        rhs=val16[:, qc],
                        start=(qc == 0),
                        stop=(qc == QT - 1),
                    )
                ctx_sb = outp.tile([P, VD], f32, tag="ctx_sb")
                nc.vector.tensor_scalar_mul(ctx_sb, ctx_ps[:, 0, 0, :VD], rec[:, kt:kt + 1])
                nc.sync.dma_start(out=ctx_d[b, kt * P:(kt + 1) * P, :], in_=ctx_sb)

        emit_planes(0)
        emit_planes(1)
        emit_softmax_ctx(1)

    nc.compile()
    return nc


def _get_nc():
    if "nc" not in _cache:
        _cache["nc"] = _build()
    return _cache["nc"]


def run_sharded(inputs, trace=False, trace_cores=None):
    from concourse.bass_utils import run_bass_kernel_spmd

    nc = _get_nc()
    q = np.ascontiguousarray(np.asarray(inputs["query"]), dtype=np.float32)
    k = np.ascontiguousarray(np.asarray(inputs["key"]), dtype=np.float32)
    val = np.ascontiguousarray(np.asarray(inputs["value"]), dtype=np.float32)
    wq = np.ascontiguousarray(np.asarray(inputs["Wq"]), dtype=np.float32)
    wk = np.ascontiguousarray(np.asarray(inputs["Wk"]), dtype=np.float32)
    vv = np.ascontiguousarray(np.asarray(inputs["v"]), dtype=np.float32)

    in_maps = []
    for c in range(N_CORES):
        sl = slice(c * B_LOC, (c + 1) * B_LOC)
        in_maps.append(
            {
                "query": np.ascontiguousarray(q[sl]),
                "key": np.ascontiguousarray(k[sl]),
                "value": np.ascontiguousarray(val[sl]),
                "Wq": wq,
                "Wk": wk,
                "v": vv,
            }
        )
    kwargs = {}
    if trace_cores is not None:
        kwargs["trace_cores"] = trace_cores
    res = run_bass_kernel_spmd(
        nc, in_maps, core_ids=list(range(N_CORES)), trace=trace, **kwargs
    )
    context = np.concatenate([r["context"] for r in res.results], axis=0)
    attention = np.concatenate([r["attention"] for r in res.results], axis=0)
    return (context, attention), res


def kernel(**inputs):
    (context, attention), _ = run_sharded(inputs, trace=False)
    return context, attention


if __name__ == "__main__":
    nc = _build()
    print("build + compile OK")


# revision 15
# speedup vs baseline: 1.1810x; 1.1191x over previous
# Additive (Bahdanau) attention kernel for Trainium2, data-parallel over batch
# on 8 NeuronCores.
#
# Per core (2 batches):
#   qp = query @ Wq            [256q, 128h]   (kept transposed: qpT [h, q])
#   kp = key @ Wk              [256k, 128h]   (kept natural:    kp  [k, h])
#   score[k, q] = sum_h v[h] * tanh(qp[q, h] + kp[k, h])
#   attention = softmax_q(score);  context = attention @ value
#
# The [k, q] plane for each h is built on the TensorEngine as two K=128 fp16
# matmuls into PSUM: an identity pass that broadcasts kp[:, h] along q, plus a
# one-hot-row pass (T128) that broadcasts qpT[h, :] along k. ScalarE applies
# tanh (fp16 out). The v-weighted reduction over h is split: even-h planes
# fold into a score PSUM bank on the TensorEngine via scaled-identity
# accumulating matmuls (VI), odd-h planes fold into two fp16 accumulators on
# VectorE (two chains so pipe drains overlap). Softmax runs per k-tile with
# Exp+accum_out (scores are bounded by sum|v| <= 6.5, so no max subtraction).
# Both batches' transposes/projections are issued up front so the PE never
# stalls at the batch boundary.

import numpy as np

B_TOTAL = 16
N_CORES = 8
B_LOC = B_TOTAL // N_CORES
L = 256      # l_q == l_k
D = 256      # q_dim == k_dim
H = 128      # attention dim
VD = 128     # value dim
P = 128      # partitions

_cache = {}


def _build():
    from contextlib import ExitStack

    import concourse.bacc as bacc
    import concourse.mybir as mybir
    import concourse.tile as tile

    f32 = mybir.dt.float32
    f16 = mybir.dt.float16
    AF = mybir.ActivationFunctionType
    OP = mybir.AluOpType

    nc = bacc.Bacc("TRN2", target_bir_lowering=False, debug=False)

    q_d = nc.dram_tensor("query", [B_LOC, L, D], f32, kind="ExternalInput")
    k_d = nc.dram_tensor("key", [B_LOC, L, D], f32, kind="ExternalInput")
    val_d = nc.dram_tensor("value", [B_LOC, L, VD], f32, kind="ExternalInput")
    wq_d = nc.dram_tensor("Wq", [D, H], f32, kind="ExternalInput")
    wk_d = nc.dram_tensor("Wk", [D, H], f32, kind="ExternalInput")
    vv_d = nc.dram_tensor("v", [H], f32, kind="ExternalInput")
    ctx_d = nc.dram_tensor("context", [B_LOC, L, VD], f32, kind="ExternalOutput")
    attn_d = nc.dram_tensor("attention", [B_LOC, L, L], f32, kind="ExternalOutput")

    KT = L // P   # k tiles (2)
    QT = L // P   # q tiles (2)
    DC = D // P   # d chunks (2)
    HH = 2        # h planes per big PSUM tile (= HH banks)

    with tile.TileContext(nc) as tc, ExitStack() as ctx:
        singles = ctx.enter_context(tc.tile_pool(name="singles", bufs=1))
        io = ctx.enter_context(tc.tile_pool(name="io", bufs=2))
        tr = ctx.enter_context(tc.tile_pool(name="tr", bufs=2))
        proj = ctx.enter_context(tc.tile_pool(name="proj", bufs=2))
        tanhp = ctx.enter_context(tc.tile_pool(name="tanhp", bufs=8))
        accp = ctx.enter_context(tc.tile_pool(name="accp", bufs=2))
        soft = ctx.enter_context(tc.tile_pool(name="soft", bufs=2))
        outp = ctx.enter_context(tc.tile_pool(name="outp", bufs=2))
        # 6 banks of plane tiles (shared with prologue/epilogue via tag "ps")
        # + 2 banks of per-batch score accumulators = all 8 PSUM banks.
        psum_big = ctx.enter_context(tc.tile_pool(name="psum_big", bufs=3, space="PSUM"))
        psum_sc = ctx.enter_context(tc.tile_pool(name="psum_sc", bufs=2, space="PSUM"))

        # --- constants -----------------------------------------------------
        # All one-hot constants via GpSimd affine_select against a broadcast
        # zero (keeps VectorE free for the score pipeline). ident32 first
        # (transposes need it), then ident16 (plane identity pass), then the
        # T128 chunks (one-hot row selectors, first needed at plane h=0).
        zero1 = singles.tile([P, 1], f16, tag="zero1")
        nc.gpsimd.memset(zero1, 0.0)
        zero1_32 = singles.tile([P, 1], f32, tag="zero1_32")
        nc.gpsimd.memset(zero1_32, 0.0)

        def onehot(dst, zsrc, base, pattern):
            z = zsrc
            while len(z.shape) < len(dst.shape):
                z = z.unsqueeze(1)
            nc.gpsimd.affine_select(
                out=dst,
                in_=z.broadcast_to(list(dst.shape)),
                compare_op=OP.not_equal,
                fill=1.0,
                base=base,
                pattern=pattern,
                channel_multiplier=1,
            )

        ident32 = singles.tile([P, P], f32, tag="ident32")
        onehot(ident32, zero1_32, 0, [[-1, P]])
        ident16 = singles.tile([P, P], f16, tag="ident16")
        onehot(ident16, zero1, 0, [[-1, P]])

        # T128[p, h, k] = 1.0 iff p == h (one-hot ROW selector; contiguous
        # slice T128[:, h, :] = fast weight load; K=128 keeps the PE array
        # fully engaged for the HAM clock governor).
        t128 = singles.tile([P, P, P], f16, tag="t128")
        for c in range(4):
            onehot(t128[:, 32 * c:32 * (c + 1), :], zero1,
                   -32 * c, [[-1, 32], [0, P]])

        vb = singles.tile([P, H], f32, tag="vb")  # v broadcast across partitions
        nc.sync.dma_start(out=vb, in_=vv_d[:].unsqueeze(0).broadcast_to([P, H]))
        vb16 = singles.tile([P, H], f16, tag="vb16")
        nc.scalar.copy(out=vb16, in_=vb)

        wq_sb = singles.tile([P, DC, H], f32, tag="wq_sb")
        nc.sync.dma_start(out=wq_sb, in_=wq_d[:].rearrange("(c p) h -> p c h", p=P))
        wk_sb = singles.tile([P, DC, H], f32, tag="wk_sb")
        nc.sync.dma_start(out=wk_sb, in_=wk_d[:].rearrange("(c p) h -> p c h", p=P))
        wq16 = singles.tile([P, DC, H], f16, tag="wq")
        nc.scalar.copy(out=wq16, in_=wq_sb)
        wk16 = singles.tile([P, DC, H], f16, tag="wk")
        nc.scalar.copy(out=wk16, in_=wk_sb)

        # --- prologue for all batches (software-pipelined) -----------------
        qpT16s, kp16s, val_sbs = [], [], []
        for b in range(B_LOC):
            q_sb = io.tile([P, QT, D], f32, tag="q_sb")
            nc.sync.dma_start(out=q_sb, in_=q_d[b].rearrange("(t p) d -> p t d", p=P))
            k_sb = io.tile([P, KT, D], f32, tag="k_sb")
            nc.sync.dma_start(out=k_sb, in_=k_d[b].rearrange("(t p) d -> p t d", p=P))
            val_sb = io.tile([P, QT, VD], f32, tag="val_sb")
            nc.sync.dma_start(out=val_sb, in_=val_d[b].rearrange("(t p) v -> p t v", p=P))
            val_sbs.append(val_sb)

            # fp32 PE transposes; fp32->fp16 conversion happens in the
            # ScalarE PSUM-evacuation copy (shortens the startup chain).
            qT16 = tr.tile([P, DC, L], f16, tag="qT")
            kT16 = tr.tile([P, DC, L], f16, tag="kT")
            for src, dst in ((q_sb, qT16), (k_sb, kT16)):
                for t in range(QT):
                    for dc in range(DC):
                        psT = psum_big.tile([P, HH, KT, L], f32, tag="ps")
                        nc.tensor.transpose(
                            psT[:, 0, 0, :P],
                            src[:, t, dc * P:(dc + 1) * P],
                            ident32,
                        )
                        nc.scalar.copy(
                            out=dst[:, dc, t * P:(t + 1) * P], in_=psT[:, 0, 0, :P]
                        )

            # qpT[h, q] = sum_d Wq[d, h] * queryT[d, q]
            qpT_ps = psum_big.tile([P, HH, KT, L], f32, tag="ps")
            for dc in range(DC):
                nc.tensor.matmul(
                    qpT_ps[:, 0, 0, :],
                    lhsT=wq16[:, dc],
                    rhs=qT16[:, dc],
                    start=(dc == 0),
                    stop=(dc == DC - 1),
                )
            qpT16 = proj.tile([P, L], f16, tag="qpT16")
            nc.scalar.copy(out=qpT16, in_=qpT_ps[:, 0, 0, :])
            qpT16s.append(qpT16)

            # kp[k, h] = sum_d keyT[d, k] * Wk[d, h]
            kp16 = proj.tile([P, KT, H], f16, tag="kp16")
            for kt in range(KT):
                kp_ps = psum_big.tile([P, HH, KT, L], f32, tag="ps")
                for dc in range(DC):
                    nc.tensor.matmul(
                        kp_ps[:, 0, 0, :H],
                        lhsT=kT16[:, dc, kt * P:(kt + 1) * P],
                        rhs=wk16[:, dc],
                        start=(dc == 0),
                        stop=(dc == DC - 1),
                    )
                nc.scalar.copy(out=kp16[:, kt, :], in_=kp_ps[:, 0, 0, :H])
            kp16s.append(kp16)

        # VI[p, j, k] = v[2j] * (p == k) for the 64 even h (= 2j) planes that
        # reduce on the TensorEngine. Chunk 0 is built before the plane loops;
        # later chunks are emitted mid-loop (emit_vi below) so they never
        # starve the VectorE stt stream.
        vi = singles.tile([P, H // 2, P], f16, tag="vi")

        def emit_vi(c):
            nc.vector.tensor_tensor(
                out=vi[:, 16 * c:16 * (c + 1), :],
                in0=ident16.unsqueeze(1).broadcast_to([P, 16, P]),
                in1=vb16[:, 32 * c:32 * (c + 1):2].unsqueeze(2).broadcast_to([P, 16, P]),
                op=OP.mult,
            )

        emit_vi(0)

        # --- main: score accumulation + softmax + context per batch --------
        state = {}

        def emit_planes(b):
            qpT16, kp16 = qpT16s[b], kp16s[b]
            acc_a = accp.tile([P, KT, L], f16, tag="acc_a")
            nc.vector.memset(acc_a, 0.0)
            acc_b = accp.tile([P, KT, L], f16, tag="acc_b")
            nc.vector.memset(acc_b, 0.0)
            score_ps = psum_sc.tile([P, KT, L], f32, tag="score")
            state[b] = (acc_a, acc_b, score_ps)

            for hp in range(H // HH):
                if b == 0 and hp in (8, 24, 40):
                    emit_vi(hp // 16 + 1)
                if b == 1 and hp == 20:
                    emit_softmax_ctx(0)
                ps = psum_big.tile([P, HH, KT, L], f32, tag="ps")
                T = tanhp.tile([P, HH, KT, L], f16, tag="T")
                for hh in range(HH):
                    h = HH * hp + hh
                    # plane[k, (kt, q)] = kp[kt*128+k, h]  (identity pass)
                    nc.tensor.matmul(
                        ps[:, hh],
                        lhsT=ident16,
                        rhs=kp16[:, :, h].unsqueeze(2).broadcast_to([P, KT, L]),
                        start=True,
                        stop=False,
                    )
                    # plane[k, (kt, q)] += qpT[h, q]  (one-hot row pass)
                    nc.tensor.matmul(
                        ps[:, hh],
                        lhsT=t128[:, h, :],
                        rhs=qpT16.unsqueeze(1).broadcast_to([P, KT, L]),
                        start=False,
                        stop=True,
                    )
                nc.scalar.activation(out=T, in_=ps, func=AF.Tanh)
                for hh in range(HH):
                    h = HH * hp + hh
                    if h % 2 == 0:
                        nc.tensor.matmul(
                            score_ps,
                            lhsT=vi[:, h // 2, :],
                            rhs=T[:, hh],
                            start=(h == 0),
                            stop=(h == H - 2),
                        )
                    else:
                        acc = acc_a if (h % 4 == 1) else acc_b
                        nc.vector.scalar_tensor_tensor(
                            out=acc,
                            in0=T[:, hh],
                            scalar=vb[:, h:h + 1],
                            in1=acc,
                            op0=OP.mult,
                            op1=OP.add,
                        )

        def emit_softmax_ctx(b):
            acc_a, acc_b, score_ps = state[b]
            val_sb = val_sbs[b]
            scoref = soft.tile([P, KT, L], f32, tag="scoref")
            nc.vector.scalar_tensor_tensor(
                out=scoref, in0=acc_a, scalar=1.0, in1=acc_b,
                op0=OP.mult, op1=OP.add,
            )
            nc.vector.tensor_tensor(
                out=scoref, in0=scoref, in1=score_ps, op=OP.add
            )

            # softmax over q (free dim); scores bounded, skip max. The exp is
            # left unnormalized for the context path (folded via rec at the
            # end); only the attention output gets the explicit 1/sum scale.
            e = soft.tile([P, KT, L], f32, tag="e")
            sums = soft.tile([P, KT], f32, tag="sums")
            rec = soft.tile([P, KT], f32, tag="rec")
            for kt in range(KT):
                nc.scalar.activation(
                    out=e[:, kt],
                    in_=scoref[:, kt],
                    func=AF.Exp,
                    accum_out=sums[:, kt:kt + 1],
                )
            nc.vector.reciprocal(rec, sums)
            attn = soft.tile([P, KT, L], f32, tag="attn")
            for kt in range(KT):
                nc.vector.tensor_scalar_mul(attn[:, kt], e[:, kt], rec[:, kt:kt + 1])
                nc.sync.dma_start(
                    out=attn_d[b, kt * P:(kt + 1) * P, :], in_=attn[:, kt]
                )

            # context = softmax(score) @ value, via fp16 transposes/matmuls on
            # the unnormalized exp; 1/sum applied on the PSUM evacuation.
            attnT = outp.tile([P, QT, L], f16, tag="attnT")
            for kt in range(KT):
                for qc in range(QT):
                    psT = psum_big.tile([P, HH, KT, L], f32, tag="ps")
                    nc.tensor.transpose(
                        psT[:, 0, 0, :P],
                        e[:, kt, qc * P:(qc + 1) * P],
                        ident32,
                    )
                    nc.scalar.copy(
                        out=attnT[:, qc, kt * P:(kt + 1) * P], in_=psT[:, 0, 0, :P]
                    )
            val16 = outp.tile([P, QT, VD], f16, tag="val16")
            nc.scalar.copy(out=val16, in_=val_sbs[b])
            for kt in range(KT):
                ctx_ps = psum_big.tile([P, HH, KT, L], f32, tag="ps")
                for qc in range(QT):
                    nc.tensor.matmul(
                        ctx_ps[:, 0, 0, :VD],
                        lhsT=attnT[:, qc, kt * P:(kt + 1) * P],
                        rhs=val16[:, qc],
                        start=(qc == 0),
                        stop=(qc == QT - 1),
                    )
                ctx_sb = outp.tile([P, VD], f32, tag="ctx_sb")
                nc.vector.tensor_scalar_mul(ctx_sb, ctx_ps[:, 0, 0, :VD], rec[:, kt:kt + 1])
                nc.sync.dma_start(out=ctx_d[b, kt * P:(kt + 1) * P, :], in_=ctx_sb)

        emit_planes(0)
        emit_planes(1)
        emit_softmax_ctx(1)

    nc.compile()
    return nc


def _get_nc():
    if "nc" not in _cache:
        _cache["nc"] = _build()
    return _cache["nc"]


def run_sharded(inputs, trace=False, trace_cores=None):
    from concourse.bass_utils import run_bass_kernel_spmd

    nc = _get_nc()
    q = np.ascontiguousarray(np.asarray(inputs["query"]), dtype=np.float32)
    k = np.ascontiguousarray(np.asarray(inputs["key"]), dtype=np.float32)
    val = np.ascontiguousarray(np.asarray(inputs["value"]), dtype=np.float32)
    wq = np.ascontiguousarray(np.asarray(inputs["Wq"]), dtype=np.float32)
    wk = np.ascontiguousarray(np.asarray(inputs["Wk"]), dtype=np.float32)
    vv = np.ascontiguousarray(np.asarray(inputs["v"]), dtype=np.float32)

    in_maps = []
    for c in range(N_CORES):
        sl = slice(c * B_LOC, (c + 1) * B_LOC)
        in_maps.append(
            {
                "query": np.ascontiguousarray(q[sl]),
                "key": np.ascontiguousarray(k[sl]),
                "value": np.ascontiguousarray(val[sl]),
                "Wq": wq,
                "Wk": wk,
                "v": vv,
            }
        )
    kwargs = {}
    if trace_cores is not None:
        kwargs["trace_cores"] = trace_cores
    res = run_bass_kernel_spmd(
        nc, in_maps, core_ids=list(range(N_CORES)), trace=trace, **kwargs
    )
    context = np.concatenate([r["context"] for r in res.results], axis=0)
    attention = np.concatenate([r["attention"] for r in res.results], axis=0)
    return (context, attention), res


def kernel(**inputs):
    (context, attention), _ = run_sharded(inputs, trace=False)
    return context, attention


if __name__ == "__main__":
    nc = _build()
    print("build + compile OK")


# revision 16
# speedup vs baseline: 1.1879x; 1.0059x over previous
# Additive (Bahdanau) attention kernel for Trainium2, data-parallel over batch
# on 8 NeuronCores.
#
# Per core (2 batches):
#   qp = query @ Wq            [256q, 128h]   (kept transposed: qpT [h, q])
#   kp = key @ Wk              [256k, 128h]   (kept natural:    kp  [k, h])
#   score[k, q] = sum_h v[h] * tanh(qp[q, h] + kp[k, h])
#   attention = softmax_q(score);  context = attention @ value
#
# The [k, q] plane for each h is built on the TensorEngine as two K=128 fp16
# matmuls into PSUM: an identity pass that broadcasts kp[:, h] along q, plus a
# one-hot-row pass (T128) that broadcasts qpT[h, :] along k. ScalarE applies
# tanh (fp16 out). The v-weighted reduction over h is split: even-h planes
# fold into a score PSUM bank on the TensorEngine via scaled-identity
# accumulating matmuls (VI), odd-h planes fold into two fp16 accumulators on
# VectorE (two chains so pipe drains overlap). Softmax runs per k-tile with
# Exp+accum_out (scores are bounded by sum|v| <= 6.5, so no max subtraction).
# Both batches' transposes/projections are issued up front so the PE never
# stalls at the batch boundary.

import numpy as np

B_TOTAL = 16
N_CORES = 8
B_LOC = B_TOTAL // N_CORES
L = 256      # l_q == l_k
D = 256      # q_dim == k_dim
H = 128      # attention dim
VD = 128     # value dim
P = 128      # partitions

_cache = {}


def _build():
    from contextlib import ExitStack

    import concourse.bacc as bacc
    import concourse.mybir as mybir
    import concourse.tile as tile

    f32 = mybir.dt.float32
    f16 = mybir.dt.float16
    AF = mybir.ActivationFunctionType
    OP = mybir.AluOpType

    nc = bacc.Bacc("TRN2", target_bir_lowering=False, debug=False)

    q_d = nc.dram_tensor("query", [B_LOC, L, D], f32, kind="ExternalInput")
    k_d = nc.dram_tensor("key", [B_LOC, L, D], f32, kind="ExternalInput")
    val_d = nc.dram_tensor("value", [B_LOC, L, VD], f32, kind="ExternalInput")
    wq_d = nc.dram_tensor("Wq", [D, H], f32, kind="ExternalInput")
    wk_d = nc.dram_tensor("Wk", [D, H], f32, kind="ExternalInput")
    vv_d = nc.dram_tensor("v", [H], f32, kind="ExternalInput")
    ctx_d = nc.dram_tensor("context", [B_LOC, L, VD], f32, kind="ExternalOutput")
    attn_d = nc.dram_tensor("attention", [B_LOC, L, L], f32, kind="ExternalOutput")

    KT = L // P   # k tiles (2)
    QT = L // P   # q tiles (2)
    DC = D // P   # d chunks (2)
    HH = 2        # h planes per big PSUM tile (= HH banks)

    with tile.TileContext(nc) as tc, ExitStack() as ctx:
        singles = ctx.enter_context(tc.tile_pool(name="singles", bufs=1))
        io = ctx.enter_context(tc.tile_pool(name="io", bufs=2))
        tr = ctx.enter_context(tc.tile_pool(name="tr", bufs=2))
        proj = ctx.enter_context(tc.tile_pool(name="proj", bufs=2))
        tanhp = ctx.enter_context(tc.tile_pool(name="tanhp", bufs=8))
        accp = ctx.enter_context(tc.tile_pool(name="accp", bufs=2))
        soft = ctx.enter_context(tc.tile_pool(name="soft", bufs=2))
        outp = ctx.enter_context(tc.tile_pool(name="outp", bufs=2))
        # 6 banks of plane tiles (shared with prologue/epilogue via tag "ps")
        # + 2 banks of per-batch score accumulators = all 8 PSUM banks.
        psum_big = ctx.enter_context(tc.tile_pool(name="psum_big", bufs=3, space="PSUM"))
        psum_sc = ctx.enter_context(tc.tile_pool(name="psum_sc", bufs=2, space="PSUM"))

        # --- constants -----------------------------------------------------
        # All one-hot constants via GpSimd affine_select against a broadcast
        # zero (keeps VectorE free for the score pipeline). ident32 first
        # (transposes need it), then ident16 (plane identity pass), then the
        # T128 chunks (one-hot row selectors, first needed at plane h=0).
        zero1 = singles.tile([P, 1], f16, tag="zero1")
        nc.gpsimd.memset(zero1, 0.0)
        zero1_32 = singles.tile([P, 1], f32, tag="zero1_32")
        nc.gpsimd.memset(zero1_32, 0.0)

        def onehot(dst, zsrc, base, pattern):
            z = zsrc
            while len(z.shape) < len(dst.shape):
                z = z.unsqueeze(1)
            nc.gpsimd.affine_select(
                out=dst,
                in_=z.broadcast_to(list(dst.shape)),
                compare_op=OP.not_equal,
                fill=1.0,
                base=base,
                pattern=pattern,
                channel_multiplier=1,
            )

        ident32 = singles.tile([P, P], f32, tag="ident32")
        onehot(ident32, zero1_32, 0, [[-1, P]])
        ident16 = singles.tile([P, P], f16, tag="ident16")
        onehot(ident16, zero1, 0, [[-1, P]])

        # T128[p, h, k] = 1.0 iff p == h (one-hot ROW selector; contiguous
        # slice T128[:, h, :] = fast weight load; K=128 keeps the PE array
        # fully engaged for the HAM clock governor).
        t128 = singles.tile([P, P, P], f16, tag="t128")
        for c in range(4):
            onehot(t128[:, 32 * c:32 * (c + 1), :], zero1,
                   -32 * c, [[-1, 32], [0, P]])

        vb = singles.tile([P, H], f32, tag="vb")  # v broadcast across partitions
        nc.sync.dma_start(out=vb, in_=vv_d[:].unsqueeze(0).broadcast_to([P, H]))
        vb16 = singles.tile([P, H], f16, tag="vb16")
        nc.scalar.copy(out=vb16, in_=vb)

        wq_sb = singles.tile([P, DC, H], f32, tag="wq_sb")
        nc.sync.dma_start(out=wq_sb, in_=wq_d[:].rearrange("(c p) h -> p c h", p=P))
        wk_sb = singles.tile([P, DC, H], f32, tag="wk_sb")
        nc.sync.dma_start(out=wk_sb, in_=wk_d[:].rearrange("(c p) h -> p c h", p=P))
        wq16 = singles.tile([P, DC, H], f16, tag="wq")
        nc.scalar.copy(out=wq16, in_=wq_sb)
        wk16 = singles.tile([P, DC, H], f16, tag="wk")
        nc.scalar.copy(out=wk16, in_=wk_sb)

        # --- prologue for all batches (software-pipelined) -----------------
        qpT16s, kp16s, val_sbs = [], [], []
        for b in range(B_LOC):
            q_sb = io.tile([P, QT, D], f32, tag="q_sb")
            nc.sync.dma_start(out=q_sb, in_=q_d[b].rearrange("(t p) d -> p t d", p=P))
            k_sb = io.tile([P, KT, D], f32, tag="k_sb")
            nc.sync.dma_start(out=k_sb, in_=k_d[b].rearrange("(t p) d -> p t d", p=P))
            val_sb = io.tile([P, QT, VD], f32, tag="val_sb")
            nc.sync.dma_start(out=val_sb, in_=val_d[b].rearrange("(t p) v -> p t v", p=P))
            val_sbs.append(val_sb)

            # fp32 PE transposes; fp32->fp16 conversion happens in the
            # ScalarE PSUM-evacuation copy (shortens the startup chain).
            qT16 = tr.tile([P, DC, L], f16, tag="qT")
            kT16 = tr.tile([P, DC, L], f16, tag="kT")
            for src, dst in ((q_sb, qT16), (k_sb, kT16)):
                for t in range(QT):
                    for dc in range(DC):
                        psT = psum_big.tile([P, HH, KT, L], f32, tag="ps")
                        nc.tensor.transpose(
                            psT[:, 0, 0, :P],
                            src[:, t, dc * P:(dc + 1) * P],
                            ident32,
                        )
                        nc.scalar.copy(
                            out=dst[:, dc, t * P:(t + 1) * P], in_=psT[:, 0, 0, :P]
                        )

            # qpT[h, q] = sum_d Wq[d, h] * queryT[d, q]
            qpT_ps = psum_big.tile([P, HH, KT, L], f32, tag="ps")
            for dc in range(DC):
                nc.tensor.matmul(
                    qpT_ps[:, 0, 0, :],
                    lhsT=wq16[:, dc],
                    rhs=qT16[:, dc],
                    start=(dc == 0),
                    stop=(dc == DC - 1),
                )
            qpT16 = proj.tile([P, L], f16, tag="qpT16")
            nc.scalar.copy(out=qpT16, in_=qpT_ps[:, 0, 0, :])
            qpT16s.append(qpT16)

            # kp[k, h] = sum_d keyT[d, k] * Wk[d, h]
            kp16 = proj.tile([P, KT, H], f16, tag="kp16")
            for kt in range(KT):
                kp_ps = psum_big.tile([P, HH, KT, L], f32, tag="ps")
                for dc in range(DC):
                    nc.tensor.matmul(
                        kp_ps[:, 0, 0, :H],
                        lhsT=kT16[:, dc, kt * P:(kt + 1) * P],
                        rhs=wk16[:, dc],
                        start=(dc == 0),
                        stop=(dc == DC - 1),
                    )
                nc.scalar.copy(out=kp16[:, kt, :], in_=kp_ps[:, 0, 0, :H])
            kp16s.append(kp16)

        # VI[p, j, k] = v[2j] * (p == k) for the 64 even h (= 2j) planes that
        # reduce on the TensorEngine. Chunk 0 is built before the plane loops;
        # later chunks are emitted mid-loop (emit_vi below) so they never
        # starve the VectorE stt stream.
        vi = singles.tile([P, H // 2, P], f16, tag="vi")

        def emit_vi(c):
            nc.vector.tensor_tensor(
                out=vi[:, 16 * c:16 * (c + 1), :],
                in0=ident16.unsqueeze(1).broadcast_to([P, 16, P]),
                in1=vb16[:, 32 * c:32 * (c + 1):2].unsqueeze(2).broadcast_to([P, 16, P]),
                op=OP.mult,
            )

        emit_vi(0)

        # --- main: score accumulation + softmax + context per batch --------
        state = {}

        def emit_planes(b):
            qpT16, kp16 = qpT16s[b], kp16s[b]
            acc_a = accp.tile([P, KT, L], f16, tag="acc_a")
            nc.vector.memset(acc_a, 0.0)
            acc_b = accp.tile([P, KT, L], f16, tag="acc_b")
            nc.vector.memset(acc_b, 0.0)
            score_ps = psum_sc.tile([P, KT, L], f32, tag="score")
            state[b] = (acc_a, acc_b, score_ps)

            for hp in range(H // HH):
                if b == 0 and hp in (8, 24, 40):
                    emit_vi(hp // 16 + 1)
                if b == 1 and hp == 20:
                    emit_softmax_ctx(0)
                ps = psum_big.tile([P, HH, KT, L], f32, tag="ps")
                T = tanhp.tile([P, HH, KT, L], f16, tag="T")
                for hh in range(HH):
                    h = HH * hp + hh
                    # plane[k, (kt, q)] = kp[kt*128+k, h]  (identity pass)
                    nc.tensor.matmul(
                        ps[:, hh],
                        lhsT=ident16,
                        rhs=kp16[:, :, h].unsqueeze(2).broadcast_to([P, KT, L]),
                        start=True,
                        stop=False,
                    )
                    # plane[k, (kt, q)] += qpT[h, q]  (one-hot row pass)
                    nc.tensor.matmul(
                        ps[:, hh],
                        lhsT=t128[:, h, :],
                        rhs=qpT16.unsqueeze(1).broadcast_to([P, KT, L]),
                        start=False,
                        stop=True,
                    )
                nc.scalar.activation(out=T, in_=ps, func=AF.Tanh)
                for hh in range(HH):
                    h = HH * hp + hh
                    if h % 2 == 0:
                        nc.tensor.matmul(
                            score_ps,
                            lhsT=vi[:, h // 2, :],
                            rhs=T[:, hh],
                            start=(h == 0),
                            stop=(h == H - 2),
                        )
                    else:
                        acc = acc_a if (h % 4 == 1) else acc_b
                        nc.vector.scalar_tensor_tensor(
                            out=acc,
                            in0=T[:, hh],
                            scalar=vb[:, h:h + 1],
                            in1=acc,
                            op0=OP.mult,
                            op1=OP.add,
                        )

        def emit_softmax_ctx(b):
            acc_a, acc_b, score_ps = state[b]
            val_sb = val_sbs[b]
            scoref = soft.tile([P, KT, L], f32, tag="scoref")
            nc.vector.scalar_tensor_tensor(
                out=scoref, in0=acc_a, scalar=1.0, in1=acc_b,
                op0=OP.mult, op1=OP.add,
            )
            nc.vector.tensor_tensor(
                out=scoref, in0=scoref, in1=score_ps, op=OP.add
            )

            # softmax over q (free dim); scores bounded, skip max. The exp is
            # left unnormalized for the context path (folded via rec at the
            # end); only the attention output gets the explicit 1/sum scale.
            e = soft.tile([P, KT, L], f32, tag="e")
            sums = soft.tile([P, KT], f32, tag="sums")
            rec = soft.tile([P, KT], f32, tag="rec")
            for kt in range(KT):
                nc.scalar.activation(
                    out=e[:, kt],
                    in_=scoref[:, kt],
                    func=AF.Exp,
                    accum_out=sums[:, kt:kt + 1],
                )
            nc.vector.reciprocal(rec, sums)
            attn = soft.tile([P, KT, L], f32, tag="attn")
            for kt in range(KT):
                nc.vector.tensor_scalar_mul(attn[:, kt], e[:, kt], rec[:, kt:kt + 1])
                nc.sync.dma_start(
                    out=attn_d[b, kt * P:(kt + 1) * P, :], in_=attn[:, kt]
                )

            # context = softmax(score) @ value, via fp16 transposes/matmuls on
            # the unnormalized exp; 1/sum applied on the PSUM evacuation.
            attnT = outp.tile([P, QT, L], f16, tag="attnT")
            for kt in range(KT):
                for qc in range(QT):
                    psT = psum_big.tile([P, HH, KT, L], f32, tag="ps")
                    nc.tensor.transpose(
                        psT[:, 0, 0, :P],
                        e[:, kt, qc * P:(qc + 1) * P],
                        ident32,
                    )
                    nc.vector.tensor_copy(
                        out=attnT[:, qc, kt * P:(kt + 1) * P], in_=psT[:, 0, 0, :P]
                    )
            val16 = outp.tile([P, QT, VD], f16, tag="val16")
            nc.scalar.copy(out=val16, in_=val_sbs[b])
            for kt in range(KT):
                ctx_ps = psum_big.tile([P, HH, KT, L], f32, tag="ps")
                for qc in range(QT):
                    nc.tensor.matmul(
                        ctx_ps[:, 0, 0, :VD],
                        lhsT=attnT[:, qc, kt * P:(kt + 1) * P],
                        rhs=val16[:, qc],
                        start=(qc == 0),
                        stop=(qc == QT - 1),
                    )
                ctx_sb = outp.tile([P, VD], f32, tag="ctx_sb")
                nc.vector.tensor_scalar_mul(ctx_sb, ctx_ps[:, 0, 0, :VD], rec[:, kt:kt + 1])
                nc.sync.dma_start(out=ctx_d[b, kt * P:(kt + 1) * P, :], in_=ctx_sb)

        emit_planes(0)
        emit_planes(1)
        emit_softmax_ctx(1)

    nc.compile()
    return nc


def _get_nc():
    if "nc" not in _cache:
        _cache["nc"] = _build()
    return _cache["nc"]


def run_sharded(inputs, trace=False, trace_cores=None):
    from concourse.bass_utils import run_bass_kernel_spmd

    nc = _get_nc()
    q = np.ascontiguousarray(np.asarray(inputs["query"]), dtype=np.float32)
    k = np.ascontiguousarray(np.asarray(inputs["key"]), dtype=np.float32)
    val = np.ascontiguousarray(np.asarray(inputs["value"]), dtype=np.float32)
    wq = np.ascontiguousarray(np.asarray(inputs["Wq"]), dtype=np.float32)
    wk = np.ascontiguousarray(np.asarray(inputs["Wk"]), dtype=np.float32)
    vv = np.ascontiguousarray(np.asarray(inputs["v"]), dtype=np.float32)

    in_maps = []
    for c in range(N_CORES):
        sl = slice(c * B_LOC, (c + 1) * B_LOC)
        in_maps.append(
            {
                "query": np.ascontiguousarray(q[sl]),
                "key": np.ascontiguousarray(k[sl]),
                "value": np.ascontiguousarray(val[sl]),
                "Wq": wq,
                "Wk": wk,
                "v": vv,
            }
        )
    kwargs = {}
    if trace_cores is not None:
        kwargs["trace_cores"] = trace_cores
    res = run_bass_kernel_spmd(
        nc, in_maps, core_ids=list(range(N_CORES)), trace=trace, **kwargs
    )
    context = np.concatenate([r["context"] for r in res.results], axis=0)
    attention = np.concatenate([r["attention"] for r in res.results], axis=0)
    return (context, attention), res


def kernel(**inputs):
    (context, attention), _ = run_sharded(inputs, trace=False)
    return context, attention


if __name__ == "__main__":
    nc = _build()
    print("build + compile OK")
